# revision 1
# baseline (speedup 1.0000x reference)
"""GroupedMLP (MoE) kernel for 8 TRN2 NeuronCores.

Expert-parallel: expert i -> core i. Each core computes, for its expert's
2048-token block X [T=2048, H=2048]:
    fc1 = X @ w1.T          # w1 [8192, 2048]
    inter = silu(a) * b     # a,b = split(fc1, 2, axis=-1)
    out = inter @ w2.T      # w2 [2048, 4096]

Device-side everything is computed transposed (contraction dim on SBUF
partitions):
  phase 1: fc1T[m,t] = sum_k w1T_tile[k,m].T @ xT[k,t]   (PSUM, fp32)
           interT = silu(fc1T[a-rows]) * fc1T[b-rows]    (bf16, SBUF)
  phase 2: yT[h,t]  = sum_f w2T_tile[f,h].T @ interT[f,t]
Matmuls run in bf16 (full-rate on the PE), accumulation in fp32 PSUM.
Two passes of 1024 tokens each so interT + x + weight tiles fit in SBUF.

Host side shards/transposes/casts inputs and transposes the output back.
"""

import numpy as np
import ml_dtypes
from contextlib import ExitStack

P = 128
H = 2048          # hidden size
F = 4096          # ffn hidden (one GLU half)
T = 2048          # tokens per expert
NE = 8            # experts == cores
TPASS = 1024      # tokens per pass
NPASS = T // TPASS
NT = 512          # matmul moving free dim (one PSUM bank of fp32)

_BF16 = ml_dtypes.bfloat16

_nc_cache = {}


def _build_nc():
    import concourse.mybir as mybir
    import concourse.tile as tile
    from concourse import bacc

    nc = bacc.Bacc("TRN2", target_bir_lowering=False, debug=False)
    bf16 = mybir.dt.bfloat16
    f32 = mybir.dt.float32
    Silu = mybir.ActivationFunctionType.Silu

    # Per-core shards, host-prearranged so every DMA is contiguous:
    #  xr[kk, p, t]        = X.T[kk*128+p, t]                       (bf16)
    #  w1r[m, p, kk, c]    = w1.T[kk*128+p, mcol(m,c)]              (bf16)
    #       mcol(m,c) = m*128+c for c<128 (silu half), 4096+m*128+(c-128) else
    #  w2r[h2, p, f, c]    = w2.T[f*128+p, h2*256+c]                (bf16)
    #  yr[hh, p, t]        = out.T[hh*128+p, t]                     (fp32)
    xr = nc.declare_dram_parameter("xr", [16, P, T], bf16, isOutput=False)
    w1r = nc.declare_dram_parameter("w1r", [32, P, 16, 256], bf16, isOutput=False)
    w2r = nc.declare_dram_parameter("w2r", [8, P, 32, 256], bf16, isOutput=False)
    yr = nc.declare_dram_parameter("yr", [16, P, T], f32, isOutput=True)

    with tile.TileContext(nc) as tc, ExitStack() as ctx:
        xpool = ctx.enter_context(tc.tile_pool(name="x", bufs=1))
        ipool = ctx.enter_context(tc.tile_pool(name="inter", bufs=1))
        w1pool = ctx.enter_context(tc.tile_pool(name="w1", bufs=3))
        w2pool = ctx.enter_context(tc.tile_pool(name="w2", bufs=2))
        tpool = ctx.enter_context(tc.tile_pool(name="tmp", bufs=2))
        opool = ctx.enter_context(tc.tile_pool(name="osb", bufs=3))
        psum = ctx.enter_context(tc.tile_pool(name="psum", bufs=2, space="PSUM"))

        # X.T resident in SBUF (8 MB bf16) as per-pass column blocks; pass-0
        # blocks load at kernel start, pass-1 blocks prefetch mid-pass-0 so
        # the first matmul isn't gated on the full 8 MB.
        xsb = [[None] * 16 for _ in range(NPASS)]

        def load_x(ps):
            off = ps * TPASS
            for kk in range(16):
                xt = xpool.tile([P, TPASS], bf16, tag=f"x{ps}_{kk}", bufs=1,
                                name=f"x{ps}_{kk}")
                nc.scalar.dma_start(xt[:], xr[kk][:, off : off + TPASS])
                xsb[ps][kk] = xt

        load_x(0)

        # HAM warmup: the PE idles ~12us at start (engine preamble + first
        # DMAs), so the clock gate sits at 1.2 GHz exactly when real matmuls
        # begin. Burn dummy matmuls on scratch data to trip the activity
        # window early; they finish before the first real operand arrives.
        warm = xpool.tile([P, NT], bf16, tag="warm", bufs=1, name="warm")
        nc.vector.memset(warm[:], 0.0)
        pw = psum.tile([P, TPASS], f32, tag="pa", name="pwarm")
        for i in range(14):
            nc.tensor.matmul(
                pw[:, 0:NT], warm[:, 0:128], warm[:], start=True, stop=True
            )

        # interT tiles: 32 x [128, TPASS] bf16 (8 MB), reused across passes.
        inter = [
            ipool.tile([P, TPASS], bf16, tag=f"i{m}", bufs=1, name=f"inter{m}")
            for m in range(32)
        ]

        for ps in range(NPASS):
            off = ps * TPASS

            # ---- phase 1: fc1T + GLU -> interT ----
            w2pre = {}
            for m in range(32):
                if ps + 1 < NPASS and m == 16:
                    load_x(ps + 1)
                if m == 2:
                    # prefetch the first two w2 tiles here (not at t=0) so
                    # startup DMA bandwidth goes to x and w1
                    for h2 in range(2):
                        w2m = w2pool.tile(
                            [P, 32, 256], bf16, tag="w2m", name=f"w2m_{ps}_{h2}"
                        )
                        nc.sync.dma_start(w2m[:], w2r[h2])
                        w2pre[h2] = w2m
                w1m = w1pool.tile([P, 16, 256], bf16, tag="w1m")
                # two half-loads so kk=0..7 matmuls can start on the first half
                nc.sync.dma_start(w1m[:, 0:8, :], w1r[m][:, 0:8, :])
                nc.sync.dma_start(w1m[:, 8:16, :], w1r[m][:, 8:16, :])
                pa = psum.tile([P, TPASS], f32, tag="pa")
                pb = psum.tile([P, TPASS], f32, tag="pb")
                for kk in range(16):
                    la = w1m[:, kk, 0:128]
                    lb = w1m[:, kk, 128:256]
                    st = kk == 0
                    sp = kk == 15
                    # consecutive matmuls share the stationary operand so
                    # walrus's ldw-opt can drop the redundant LDWEIGHTS
                    for lhs, pd in ((la, pa), (lb, pb)):
                        for tb in range(TPASS // NT):
                            r = xsb[ps][kk][:, tb * NT : (tb + 1) * NT]
                            nc.tensor.matmul(
                                pd[:, tb * NT : (tb + 1) * NT], lhs, r,
                                start=st, stop=sp,
                            )
                tmp = tpool.tile([P, TPASS], f32, tag="tmp")
                nc.scalar.activation(tmp[:], pa[:], Silu)
                nc.vector.tensor_mul(inter[m][:], tmp[:], pb[:])

            # ---- phase 2: yT = w2T.T @ interT ----
            for h2 in range(8):
                if h2 in w2pre:
                    w2m = w2pre[h2]
                else:
                    w2m = w2pool.tile([P, 32, 256], bf16, tag="w2m")
                    nc.sync.dma_start(w2m[:], w2r[h2])
                for hh in range(2):
                    po = psum.tile([P, TPASS], f32, tag="pa")  # reuse pa slots
                    for f in range(32):
                        lw = w2m[:, f, hh * 128 : (hh + 1) * 128]
                        st = f == 0
                        sp = f == 31
                        for tb in range(TPASS // NT):
                            nc.tensor.matmul(
                                po[:, tb * NT : (tb + 1) * NT],
                                lw,
                                inter[f][:, tb * NT : (tb + 1) * NT],
                                start=st,
                                stop=sp,
                            )
                    osb = opool.tile([P, TPASS], f32, tag="osb")
                    # copy on DVE (idle in phase 2) so ScalarE never swaps
                    # activation tables; split halves to overlap copy and store
                    for tb in range(TPASS // NT):
                        sl = slice(tb * NT, (tb + 1) * NT)
                        nc.vector.tensor_copy(osb[:, sl], po[:, sl])
                        nc.scalar.dma_start(
                            yr[h2 * 2 + hh][:, off + tb * NT : off + (tb + 1) * NT],
                            osb[:, sl],
                        )
    nc.compile()
    return nc


def _prep_core_inputs(x, w1_i, w2_i):
    """Host-side reshape/cast of one expert's shard into DMA-friendly layouts."""
    xT = np.ascontiguousarray(x.T)                       # [H, T]
    xr = xT.reshape(16, P, T).astype(_BF16)

    w1T = w1_i.T                                         # [H, 8192]
    a = w1T[:, :F].reshape(H, 32, P)
    b = w1T[:, F:].reshape(H, 32, P)
    cat = np.concatenate([a, b], axis=2)                 # [H, 32, 256]
    w1r = np.ascontiguousarray(
        cat.reshape(16, P, 32, 256).transpose(2, 1, 0, 3)
    ).astype(_BF16)                                      # [32, P, 16, 256]

    w2T = w2_i.T                                         # [F, H]
    w2r = np.ascontiguousarray(
        w2T.reshape(32, P, 8, 256).transpose(2, 1, 0, 3)
    ).astype(_BF16)                                      # [8, P, 32, 256]
    return {"xr": xr, "w1r": w1r, "w2r": w2r}


_last_results = None


def kernel(permuted_hidden_states, tokens_per_expert, w1, w2):
    global _last_results
    x = np.asarray(permuted_hidden_states, dtype=np.float32)
    counts = np.asarray(tokens_per_expert).astype(np.int64)
    w1 = np.asarray(w1, dtype=np.float32)
    w2 = np.asarray(w2, dtype=np.float32)

    if not (counts.shape == (NE,) and np.all(counts == T)):
        return _numpy_fallback(x, counts, w1, w2)

    from concourse.bass_utils import run_bass_kernel_spmd

    if "nc" not in _nc_cache:
        _nc_cache["nc"] = _build_nc()
    nc = _nc_cache["nc"]

    in_maps = [
        _prep_core_inputs(x[i * T : (i + 1) * T], w1[i], w2[i]) for i in range(NE)
    ]
    import os

    res = run_bass_kernel_spmd(
        nc,
        in_maps,
        core_ids=list(range(NE)),
        trace=bool(os.environ.get("BASS_TRACE")),
    )
    _last_results = res

    out = np.empty((NE * T, H), dtype=np.float32)
    for i in range(NE):
        yT = res.results[i]["yr"].reshape(H, T)
        out[i * T : (i + 1) * T] = yT.T
    return out


def _numpy_fallback(x, counts, w1, w2):
    outs = []
    start = 0
    for i in range(counts.shape[0]):
        n = int(counts[i])
        if n == 0:
            continue
        xi = x[start : start + n]
        fc1 = xi @ w1[i].T
        a, b = fc1[:, :F], fc1[:, F:]
        inter = (a / (1.0 + np.exp(-a))) * b
        outs.append(inter @ w2[i].T)
        start += n
    return np.concatenate(outs, axis=0).astype(np.float32)

